# revision 15
# baseline (speedup 1.0000x reference)
"""Trainium2 Bass kernel for the Dial2vec contrastive loss (nn_Dial2vec).

Math: the dense reference computes, per sequence,
    q = h * a[:,None]; r = h * b[:,None]               (a/b = role-0/1 masks)
    w = q @ r^T; fw = w * band                         (band from turn ids)
    q_cross = fw^T @ q; r_cross = fw @ r
then masked means of q / q_cross / r / r_cross, cosine similarities, and a
label-weighted log-softmax loss.

Because band[i,j] depends only on (turn_i, turn_j) and a*b == 0, everything
collapses to per-turn segment sums over the 16 turns:
    Q_T[t] = sum_{turn_l = t} a_l h_l;  R_T[t] likewise with b     [16, H]
    gam_l  = a_l (Band R_T)[turn_l].h_l + b_l (Band Q_T)[turn_l].h_l
    qs = sum a_l h_l; qc = sum a_l gam_l h_l; rs/rc likewise with b
and cosine similarity is scale-invariant, so the mask-count denominators
cancel and gam can carry an arbitrary power-of-two scale (1/16 here, to fit
fp8 range).

Device pipeline per 3-sequence group (data parallel over 8 cores, one
dialogue = 10 sequences per core, fp8 activations with fp32 PSUM accumulate):
  A  : QRT[32,H] = [A1|B1]^T @ h          (token contraction, PE col-tiled)
  T  : QRT^T via one xbar DMA-transpose   (bf16, SBUF->SBUF)
  Y' : Y[32,LC] = QRT @ h^T               (H contraction vs host-shipped
                                           H-major h, PE col-tiled)
  Z  : Z = abx . Y'                       (one small DVE multiply)
  y  : per-token gam-mask cols = Z^T @ ones/16   (one tiny PE matmul/chunk)
  D  : [qs,rs,qc,rc] = [a,b,y_a,y_b]^T @ h (token contraction, PE col-tiled)
The host performs index-only preprocessing (one-hot / band-smeared masks,
fp8 casts, both h layouts) and the final O(B*H) cosine/log-softmax
reduction over the 40 gathered fp32 vectors per core.
"""

import os

import numpy as np

B_SEQ = 80
L = 512
H = 768
SAMPLES = 10
T = 16
VIEW_RANGE = 2
TEMP = 0.2
AVG_EPS = 1e-6
COS_EPS = 1e-8

N_CORES = 8
SPC = SAMPLES  # sequences per core = one dialogue
P = 128
LC = 384  # compacted token count (attention_mask=1 tokens only, zero-padded)
CHUNKS = LC // P  # 3
HS = H // P  # 6 H-slices
N_SPLITS = ((0, 512), (512, 768))  # PSUM-bank-aligned fp32 free-dim splits
SC = 1.0 / 16  # gam scale (power of two; cancels in cosine)

# 2T-row supergroups of 3 sequences (PSUM base partitions {0,32,64})
GROUPS = [list(range(g, min(g + 3, SPC))) for g in range(0, SPC, 3)]
NG = len(GROUPS)
FSEQ = 2 * CHUNKS * H  # per-seq hh cols: token-major hid | H-major hidT
DCW = 3 * 2 * T + 4 * 3 * (CHUNKS - 1)  # dcg cols per group: 96 + 12 + 12

_CACHE: dict = {}


def _build_nc(repeat: int = 1):
    """Build + compile the per-core Bass program (identical on all cores)."""
    from contextlib import ExitStack

    import concourse.bacc as bacc
    import concourse.mybir as mybir
    import concourse.tile as tile

    f32 = mybir.dt.float32
    bf16 = mybir.dt.bfloat16
    f8 = mybir.dt.float8e4

    nc = bacc.Bacc(
        "TRN2",
        debug=False,
        enable_asserts=False,
        target_bir_lowering=False,
    )

    # partition-major layouts: a group load reads one contiguous
    # [128, G*F] block per partition -> large DMA descriptors
    hh = nc.dram_tensor("hh", [P, SPC, FSEQ], f8, kind="ExternalInput").ap()
    ab = nc.dram_tensor("ab", [P, SPC, CHUNKS * 2 * T], f8, kind="ExternalInput").ap()
    abx = nc.dram_tensor("abx", [SPC, 2 * T, LC], bf16, kind="ExternalInput").ap()
    dcg = nc.dram_tensor("dcg", [NG, P, DCW], f8, kind="ExternalInput").ap()
    ones = nc.dram_tensor("ones", [3 * 2 * T, 6], bf16, kind="ExternalInput").ap()
    out = nc.dram_tensor("out", [4 * SPC, H], f32, kind="ExternalOutput").ap()

    with tile.TileContext(nc) as tc, ExitStack() as ctx:
        hhp = ctx.enter_context(tc.tile_pool(name="hhp", bufs=NG))
        abp = ctx.enter_context(tc.tile_pool(name="abp", bufs=NG))
        axp = ctx.enter_context(tc.tile_pool(name="axp", bufs=NG))
        dcp = ctx.enter_context(tc.tile_pool(name="dcp", bufs=NG))
        sap = ctx.enter_context(tc.tile_pool(name="sap", bufs=2))
        tp = ctx.enter_context(tc.tile_pool(name="tp", bufs=2))
        t8p = ctx.enter_context(tc.tile_pool(name="t8p", bufs=2))
        zp = ctx.enter_context(tc.tile_pool(name="zp", bufs=2))
        osp = ctx.enter_context(tc.tile_pool(name="osp", bufs=2))
        onp = ctx.enter_context(tc.tile_pool(name="onp", bufs=1))
        pps = ctx.enter_context(tc.tile_pool(name="pps", bufs=4, space="PSUM"))

        onest = onp.tile([3 * 2 * T, 6], bf16, name="ones", tag="on")
        nc.gpsimd.dma_start(onest[:], ones)

        for rep in range(repeat):
            st: dict = {}

            def emit_loads(gi, split_hid=False):
                grp = GROUPS[gi]
                G = len(grp)
                s0 = grp[0]
                GP = 32 * G
                hhg = hhp.tile([P, 3 * FSEQ], f8, name=f"hh{rep}_{gi}", tag="hh")
                hhv = hhg[:, 0 : G * FSEQ].rearrange("p (s f) -> p s f", s=G)
                # small stage-A/D weights ride the SWDGE (gpsimd) queue so the
                # sync queue stays dedicated to the big h streams + transposes
                abg = abp.tile(
                    [P, 3 * CHUNKS * 2 * T], f8, name=f"ab{rep}_{gi}", tag="ab"
                )
                nc.gpsimd.dma_start(
                    abg[:, 0 : G * CHUNKS * 2 * T].rearrange("p (s f) -> p s f", s=G),
                    ab[:, s0 : s0 + G, :],
                )
                # split hid / hidT halves: stage A only waits on the first
                if split_hid:
                    for j in range(G):
                        nc.sync.dma_start(
                            hhv[:, j, 0 : CHUNKS * H],
                            hh[:, s0 + j, 0 : CHUNKS * H],
                        )
                else:
                    nc.sync.dma_start(
                        hhv[:, :, 0 : CHUNKS * H],
                        hh[:, s0 : s0 + G, 0 : CHUNKS * H],
                    )
                nc.sync.dma_start(
                    hhv[:, :, CHUNKS * H : FSEQ],
                    hh[:, s0 : s0 + G, CHUNKS * H : FSEQ],
                )
                axg = axp.tile([3 * 2 * T, LC], bf16, name=f"ax{rep}_{gi}", tag="ax")
                nc.gpsimd.dma_start(
                    axg[0:GP, :],
                    abx[s0 : s0 + G].rearrange("g t l -> (g t) l"),
                )
                dct = dcp.tile([P, DCW], f8, name=f"dc{rep}_{gi}", tag="dc")
                nc.gpsimd.dma_start(dct[:], dcg[gi])
                st[gi] = {"hhg": hhg, "abg": abg, "axg": axg, "dct": dct}

            def hid(gi, j, c, n0=0, n1=H):
                hhg = st[gi]["hhg"]
                return hhg[:, FSEQ * j + H * c + n0 : FSEQ * j + H * c + n1]

            def hidT(gi, j, k):
                hhg = st[gi]["hhg"]
                base = FSEQ * j + CHUNKS * H + LC * k
                return hhg[:, base : base + LC]

            def emit_A(gi):
                # stage A: QRT = [A1|B1]^T @ h (PE, col-tiled over seqs)
                grp = GROUPS[gi]
                G = len(grp)
                GP = 32 * G
                abg = st[gi]["abg"]
                pA = pps.tile([P, H], f32, name=f"pA{rep}_{gi}", tag="p")
                for j in range(G):
                    for c in range(CHUNKS):
                        lw = abg[:, 96 * j + 32 * c : 96 * j + 32 * c + 32]
                        for n0, n1 in N_SPLITS:
                            nc.tensor.matmul(
                                pA[32 * j : 32 * j + 32, n0:n1],
                                lw,
                                hid(gi, j, c, n0, n1),
                                start=(c == 0),
                                stop=(c == CHUNKS - 1),
                            )
                # QRT -> bf16 SBUF (DVE) -> QRT^T via one xbar DMA-transpose
                sbA = sap.tile([3 * 2 * T, H], bf16, name=f"sA{rep}_{gi}", tag="sa")
                nc.vector.tensor_copy(sbA[0:GP, :], pA[0:GP, :])
                tg = tp.tile([P, HS * 3 * 2 * T], bf16, name=f"tg{rep}_{gi}", tag="tg")
                tgv = tg[:].rearrange("p (k c) -> p k c", k=HS)
                nc.sync.dma_start(tgv[:, :, 0:GP], sbA[0:GP, :], transpose=True)
                st[gi]["tg"] = tg

            def emit_Y(gi):
                # cast QRT^T to fp8, then Y' = QRT @ h^T (PE, col-tiled)
                grp = GROUPS[gi]
                G = len(grp)
                GP = 32 * G
                tgv = st[gi]["tg"][:].rearrange("p (k c) -> p k c", k=HS)
                t8 = t8p.tile([P, HS * 3 * 2 * T], f8, name=f"t8{rep}_{gi}", tag="t8")
                t8v = t8[:].rearrange("p (k c) -> p k c", k=HS)
                nc.vector.tensor_copy(t8v[:, :, 0:GP], tgv[:, :, 0:GP])
                pY = pps.tile([P, H], f32, name=f"pY{rep}_{gi}", tag="p")
                for j in range(G):
                    for k in range(HS):
                        nc.tensor.matmul(
                            pY[32 * j : 32 * j + 32, 0:LC],
                            t8[:, 96 * k + 32 * j : 96 * k + 32 * j + 32],
                            hidT(gi, j, k),
                            start=(k == 0),
                            stop=(k == HS - 1),
                        )
                st[gi]["pY"] = pY

            def emit_Zy(gi):
                # Z = abx . Y' (DVE); y cols = Z^T @ ones/16 (PE); scatter (ACT)
                grp = GROUPS[gi]
                G = len(grp)
                GP = 32 * G
                pY = st[gi]["pY"]
                axg = st[gi]["axg"]
                dct = st[gi]["dct"]
                zt = zp.tile([3 * 2 * T, LC], bf16, name=f"z{rep}_{gi}", tag="z")
                nc.vector.tensor_mul(zt[0:GP, :], pY[0:GP, 0:LC], axg[0:GP, :])
                for c in range(CHUNKS):
                    nc.tensor.matmul(
                        pY[:, 512 + 6 * c : 512 + 6 * c + 6],
                        zt[0:GP, 128 * c : 128 * c + 128],
                        onest[0:GP, :],
                        start=True,
                        stop=True,
                        skip_group_check=True,
                    )
                for c in range(CHUNKS):
                    pyv = pY[:, 512 + 6 * c : 512 + 6 * c + 6].rearrange(
                        "p (s q) -> p s q", q=2
                    )
                    if c == 0:
                        dv = dct[:, 0 : 32 * G].rearrange("p (s q) -> p s q", q=32)
                    else:
                        o = 3 * 2 * T + 12 * (c - 1)
                        dv = dct[:, o : o + 4 * G].rearrange("p (s q) -> p s q", q=4)
                    nc.scalar.copy(dv[:, :, 2:4], pyv[:, 0:G, :])

            def emit_D(gi):
                # stage D: [qs,rs,qc,rc] = [a,b,ya,yb]^T @ h (PE, col-tiled)
                grp = GROUPS[gi]
                G = len(grp)
                GP = 32 * G
                dct = st[gi]["dct"]
                pD = pps.tile([P, H], f32, name=f"pD{rep}_{gi}", tag="p")
                for j in range(G):
                    for c in range(CHUNKS):
                        if c == 0:
                            rows = 2 * T
                            lw = dct[:, 32 * j : 32 * j + 32]
                        else:
                            rows = 4
                            o = 3 * 2 * T + 12 * (c - 1)
                            lw = dct[:, o + 4 * j : o + 4 * j + 4]
                        for n0, n1 in N_SPLITS:
                            nc.tensor.matmul(
                                pD[32 * j : 32 * j + rows, n0:n1],
                                lw,
                                hid(gi, j, c, n0, n1),
                                start=(c == 0),
                                stop=(c == CHUNKS - 1),
                                skip_group_check=True,
                            )
                osb = osp.tile([3 * 2 * T, H], f32, name=f"o{rep}_{gi}", tag="o")
                nc.scalar.copy(osb[0:GP, :], pD[0:GP, :])
                # outputs ride the scalar HWDGE queue (naturally ordered
                # right after the copy, keeps the sync queue for loads)
                for j, s in enumerate(grp):
                    nc.scalar.dma_start(
                        out[4 * s : 4 * s + 4, :], osb[32 * j : 32 * j + 4, :]
                    )

            # all loads issue up front (sync queue never blocks on compute
            # sems); the software pipeline keeps the PE fed with group g+1's
            # stage A while group g's transpose round-trips through DMA
            for gi in range(NG):
                emit_loads(gi, split_hid=(gi == 0))
            emit_A(0)
            for gi in range(NG):
                if gi + 1 < NG:
                    emit_A(gi + 1)
                emit_Y(gi)
                emit_Zy(gi)
                emit_D(gi)

    nc.compile()
    return nc


def _prep_core_inputs(hidden_states, attention_mask, role_ids, turn_ids):
    """Per-core input maps: one-hot / band-smeared mask prep (index work only)."""
    import ml_dtypes

    bf16 = ml_dtypes.bfloat16
    f8 = ml_dtypes.float8_e4m3

    active = attention_mask != 0
    counts = active.sum(-1)
    assert counts.max() <= LC, f"active tokens {counts.max()} exceed LC={LC}"
    # stable-sort active tokens to the front, keep the first LC positions.
    # Padded positions carry real h values but zero masks, so every
    # contribution they could make is exactly zero.
    sel = np.argsort(~active, axis=1, kind="stable")[:, :LC]  # [B, LC]

    am = np.take_along_axis(active, sel, axis=1).astype(np.float32)
    ro = np.take_along_axis(role_ids, sel, axis=1)
    tu = np.take_along_axis(turn_ids, sel, axis=1)
    hidden_states = np.take_along_axis(hidden_states, sel[..., None], axis=1)

    a = am * (ro == 0)
    b = am * (ro == 1)
    onehot = (tu[..., None] == np.arange(T, dtype=tu.dtype)).astype(
        np.float32
    )  # [B, LC, T]
    A1 = onehot * a[..., None]
    B1 = onehot * b[..., None]
    band = (
        np.abs(np.arange(T)[:, None] - np.arange(T)[None, :]) <= VIEW_RANGE
    ).astype(np.float32)
    A1b = A1 @ band  # a_l * band[turn_l, :]
    B1b = B1 @ band

    h8 = hidden_states.astype(f8)  # [B, LC, H]
    # token-major: [B, 128, CHUNKS*H]
    hid = (
        h8.reshape(B_SEQ, CHUNKS, P, H).transpose(0, 2, 1, 3).reshape(B_SEQ, P, -1)
    )
    # H-major: [B, 128, HS*LC]
    hidT = (
        h8.transpose(0, 2, 1)
        .reshape(B_SEQ, HS, P, LC)
        .transpose(0, 2, 1, 3)
        .reshape(B_SEQ, P, -1)
    )
    hh = np.concatenate([hid, hidT], axis=-1)  # [B, 128, FSEQ]

    # stage-A weights: per chunk the [A1(16) | B1(16)] columns
    ab_full = (
        np.concatenate([A1, B1], axis=-1)
        .reshape(B_SEQ, CHUNKS, P, 2 * T)
        .transpose(0, 2, 1, 3)
        .reshape(B_SEQ, P, -1)
    ).astype(f8)

    def core_pmajor(x, c):
        # [SPC, P, F] core slice -> partition-major [P, SPC, F]
        return np.ascontiguousarray(
            x[c * SPC : (c + 1) * SPC].transpose(1, 0, 2)
        )

    abx = np.ascontiguousarray(
        np.concatenate([B1b, A1b], axis=-1).transpose(0, 2, 1)
    ).astype(bf16)  # [B, 2T, LC]

    # stage-D weight template per group: [a, b, 0, 0] cols; chunk 0 padded
    # to 32 cols so stage D's start=True initializes the full psum strip
    ab2 = np.stack([a, b], axis=-1).reshape(B_SEQ, CHUNKS, P, 2)
    dcg_all = np.zeros((N_CORES, NG, P, DCW), np.float32)
    for gi, grp in enumerate(GROUPS):
        for j, sj in enumerate(grp):
            for core in range(N_CORES):
                s = core * SPC + sj
                dcg_all[core, gi, :, 32 * j : 32 * j + 2] = ab2[s, 0]
                for c in range(1, CHUNKS):
                    o = 3 * 2 * T + 12 * (c - 1) + 4 * j
                    dcg_all[core, gi, :, o : o + 2] = ab2[s, c]

    # ones/16 pattern: for seq strip j, rows 32j..32j+16 are the b*gam half
    # (abx rows B1b x Y'_Q), rows 32j+16..32j+32 the a*gam half.
    # dct col order is [a, b, a*gam, b*gam] -> py col 2j = a*gam, 2j+1 = b*gam
    ones3 = np.zeros((3 * 2 * T, 6), np.float32)
    for j in range(3):
        ones3[32 * j + T : 32 * j + 2 * T, 2 * j] = SC  # a*gam
        ones3[32 * j : 32 * j + T, 2 * j + 1] = SC  # b*gam
    ones3 = ones3.astype(bf16)

    in_maps = []
    for c in range(N_CORES):
        sl = slice(c * SPC, (c + 1) * SPC)
        in_maps.append(
            {
                "hh": core_pmajor(hh, c),
                "ab": core_pmajor(ab_full, c),
                "abx": np.ascontiguousarray(abx[sl]),
                "dcg": np.ascontiguousarray(dcg_all[c]).astype(f8),
                "ones": ones3,
            }
        )
    # cheap reference for a device-integrity check: qs/rs rows only,
    # recomputed on host in fp32 from the same fp8 values
    hf = h8.astype(np.float32)
    qs_ref = np.einsum("bl,blh->bh", a, hf)
    rs_ref = np.einsum("bl,blh->bh", b, hf)
    return in_maps, a.sum(-1), b.sum(-1), qs_ref, rs_ref


def _outputs_ok(outs, qs_ref, rs_ref):
    """Detect corrupted device runs: finite outputs + stage-D qs/rs match host."""
    vecs = np.concatenate(outs, axis=0).reshape(-1, 4, H)
    if not np.isfinite(vecs).all():
        return False
    for got, ref in ((vecs[:, 0], qs_ref), (vecs[:, 1], rs_ref)):
        num = np.linalg.norm(got - ref, axis=-1)
        den = np.linalg.norm(ref, axis=-1) + 1e-6
        if (num / den).max() > 0.05:
            return False
    return True


def _finalize(outs, labels, na, nb):
    """Host-side O(B*H) reduction: cosine, log-softmax, label-weighted loss."""
    vecs = np.concatenate(outs, axis=0).astype(np.float64).reshape(-1, 4, H)
    qs = vecs[:, 0] / (na + AVG_EPS)[:, None]
    rs = vecs[:, 1] / (nb + AVG_EPS)[:, None]
    qc = vecs[:, 2] / (nb + AVG_EPS)[:, None]  # carries the SC scale: cancels
    rc = vecs[:, 3] / (na + AVG_EPS)[:, None]

    def cos(x, y):
        nx = np.maximum(np.linalg.norm(x, axis=-1), COS_EPS)
        ny = np.maximum(np.linalg.norm(y, axis=-1), COS_EPS)
        return (x * y).sum(-1) / (nx * ny)

    logit_q = (cos(qs, qc) / TEMP).reshape(-1, SAMPLES)
    logit_r = (cos(rs, rc) / TEMP).reshape(-1, SAMPLES)

    def lsm(x):
        m = x.max(-1, keepdims=True)
        e = np.exp(x - m)
        return x - m - np.log(e.sum(-1, keepdims=True))

    lab = labels.astype(np.float64)
    loss_q = -np.mean(lsm(logit_q) * lab)
    loss_r = -np.mean(lsm(logit_r) * lab)
    return np.float32(loss_r + loss_q)


def kernel(hidden_states, labels, attention_mask, role_ids, turn_ids):
    import time

    from concourse.bass_utils import run_bass_kernel_spmd

    if "nc" not in _CACHE:
        _CACHE["nc"] = _build_nc()
    nc = _CACHE["nc"]

    in_maps, na, nb, qs_ref, rs_ref = _prep_core_inputs(
        np.asarray(hidden_states),
        np.asarray(attention_mask),
        np.asarray(role_ids),
        np.asarray(turn_ids),
    )
    trace = bool(os.environ.get("BASS_KERNEL_TRACE"))

    # the axon/NRT path very occasionally drops a run (device-unrecoverable
    # or corrupted output); validate cheaply and retry rather than fail
    outs = None
    for attempt in range(3):
        try:
            res = run_bass_kernel_spmd(
                nc, in_maps, core_ids=list(range(N_CORES)), trace=trace
            )
            cand = [res.results[c]["out"] for c in range(N_CORES)]
        except Exception:
            if attempt == 2:
                raise
            time.sleep(2.0)
            continue
        outs = cand
        if _outputs_ok(cand, qs_ref, rs_ref):
            break
    if trace:
        _CACHE["last_results"] = res
        print(
            f"[kernel] exec_time_ns={res.exec_time_ns} "
            f"mean_exec_time_ns={res.mean_exec_time_ns}"
        )
    return _finalize(outs, np.asarray(labels), na, nb)


# revision 20
# speedup vs baseline: 1.1677x; 1.1677x over previous
"""Trainium2 Bass kernel for the Dial2vec contrastive loss (nn_Dial2vec).

Math: the dense reference computes, per sequence,
    q = h * a[:,None]; r = h * b[:,None]               (a/b = role-0/1 masks)
    w = q @ r^T; fw = w * band                         (band from turn ids)
    q_cross = fw^T @ q; r_cross = fw @ r
then masked means of q / q_cross / r / r_cross, cosine similarities, and a
label-weighted log-softmax loss.

Because band[i,j] depends only on (turn_i, turn_j) and a*b == 0, everything
collapses to per-turn segment sums over the 16 turns:
    Q_T[t] = sum_{turn_l = t} a_l h_l;  R_T[t] likewise with b     [16, H]
    gam_l  = a_l (Band R_T)[turn_l].h_l + b_l (Band Q_T)[turn_l].h_l
    qs = sum a_l h_l; qc = sum a_l gam_l h_l; rs/rc likewise with b
and cosine similarity is scale-invariant, so the mask-count denominators
cancel and gam can carry an arbitrary power-of-two scale (1/16 here, to fit
fp8 range).

Device pipeline per 3-sequence group (data parallel over 8 cores, one
dialogue = 10 sequences per core, fp8 activations with fp32 PSUM accumulate):
  A  : QRT[32,H] = [A1|B1]^T @ h          (token contraction, PE col-tiled)
  T  : QRT^T via one xbar DMA-transpose   (bf16, SBUF->SBUF)
  Y' : Y[32,LC] = QRT @ h^T               (H contraction vs host-shipped
                                           H-major h, PE col-tiled)
  Z  : Z = abx . Y'                       (one small DVE multiply)
  y  : per-token gam-mask cols = Z^T @ ones/16   (one tiny PE matmul/chunk)
  D  : [qs,rs,qc,rc] = [a,b,y_a,y_b]^T @ h (token contraction, PE col-tiled)
The host performs index-only preprocessing (one-hot / band-smeared masks,
fp8 casts, both h layouts) and the final O(B*H) cosine/log-softmax
reduction over the 40 gathered fp32 vectors per core.
"""

import os

import numpy as np

B_SEQ = 80
L = 512
H = 768
SAMPLES = 10
T = 16
VIEW_RANGE = 2
TEMP = 0.2
AVG_EPS = 1e-6
COS_EPS = 1e-8

N_CORES = 8
SPC = SAMPLES  # sequences per core = one dialogue
P = 128
LC = 384  # compacted token count (attention_mask=1 tokens only, zero-padded)
CHUNKS = LC // P  # 3
HS = H // P  # 6 H-slices
N_SPLITS = ((0, 512), (512, 768))  # PSUM-bank-aligned fp32 free-dim splits
SC = 1.0 / 16  # gam scale (power of two; cancels in cosine)

# 2T-row supergroups of 3 sequences (PSUM base partitions {0,32,64})
GROUPS = [list(range(g, min(g + 3, SPC))) for g in range(0, SPC, 3)]
NG = len(GROUPS)
FSEQ = 2 * CHUNKS * H  # per-seq hh cols: token-major hid | H-major hidT
DCW = 3 * 2 * T + 4 * 3 * (CHUNKS - 1)  # dcg cols per group: 96 + 12 + 12

_CACHE: dict = {}


def _build_nc(repeat: int = 1):
    """Build + compile the per-core Bass program (identical on all cores)."""
    from contextlib import ExitStack

    import concourse.bacc as bacc
    import concourse.mybir as mybir
    import concourse.tile as tile

    f32 = mybir.dt.float32
    bf16 = mybir.dt.bfloat16
    f8 = mybir.dt.float8e4

    nc = bacc.Bacc(
        "TRN2",
        debug=False,
        enable_asserts=False,
        target_bir_lowering=False,
    )

    # partition-major layouts: a group load reads one contiguous
    # [128, G*F] block per partition -> large DMA descriptors.
    # hh dim 1: 0 = token-major hid, 1 = H-major hidT
    hh = nc.dram_tensor("hh", [P, 2, SPC, CHUNKS * H], f8, kind="ExternalInput").ap()
    ab = nc.dram_tensor("ab", [P, SPC, CHUNKS * 2 * T], f8, kind="ExternalInput").ap()
    abx = nc.dram_tensor("abx", [SPC, 2 * T, LC], bf16, kind="ExternalInput").ap()
    dcg = nc.dram_tensor("dcg", [NG, P, DCW], f8, kind="ExternalInput").ap()
    ones = nc.dram_tensor("ones", [3 * 2 * T, 6], bf16, kind="ExternalInput").ap()
    out = nc.dram_tensor("out", [4 * SPC, H], f32, kind="ExternalOutput").ap()

    with tile.TileContext(nc) as tc, ExitStack() as ctx:
        hhp = ctx.enter_context(tc.tile_pool(name="hhp", bufs=NG))
        abp = ctx.enter_context(tc.tile_pool(name="abp", bufs=NG))
        axp = ctx.enter_context(tc.tile_pool(name="axp", bufs=NG))
        dcp = ctx.enter_context(tc.tile_pool(name="dcp", bufs=NG))
        sap = ctx.enter_context(tc.tile_pool(name="sap", bufs=2))
        tp = ctx.enter_context(tc.tile_pool(name="tp", bufs=2))
        t8p = ctx.enter_context(tc.tile_pool(name="t8p", bufs=2))
        zp = ctx.enter_context(tc.tile_pool(name="zp", bufs=2))
        osp = ctx.enter_context(tc.tile_pool(name="osp", bufs=2))
        onp = ctx.enter_context(tc.tile_pool(name="onp", bufs=1))
        pps = ctx.enter_context(tc.tile_pool(name="pps", bufs=4, space="PSUM"))

        onest = onp.tile([3 * 2 * T, 6], bf16, name="ones", tag="on")
        nc.gpsimd.dma_start(onest[:], ones)

        for rep in range(repeat):
            st: dict = {}

            def emit_loads(gi):
                grp = GROUPS[gi]
                G = len(grp)
                s0 = grp[0]
                GP = 32 * G
                HW2 = CHUNKS * H  # per-seq half width (2304)
                hhg = hhp.tile([P, 3 * FSEQ], f8, name=f"hh{rep}_{gi}", tag="hh")
                # small stage-A/D weights ride the SWDGE (gpsimd) queue so the
                # HWDGE queues stay dedicated to the big h streams
                abg = abp.tile(
                    [P, 3 * CHUNKS * 2 * T], f8, name=f"ab{rep}_{gi}", tag="ab"
                )
                nc.gpsimd.dma_start(
                    abg[:, 0 : G * CHUNKS * 2 * T].rearrange("p (s f) -> p s f", s=G),
                    ab[:, s0 : s0 + G, :],
                )
                # hid half on the sync HWDGE ring, hidT half on the scalar
                # HWDGE ring: contiguous [128, G*2304] blocks on both sides
                nc.sync.dma_start(
                    hhg[:, 0 : G * HW2],
                    hh[:, 0, s0 : s0 + G, :].rearrange("p s f -> p (s f)"),
                )
                nc.scalar.dma_start(
                    hhg[:, G * HW2 : 2 * G * HW2],
                    hh[:, 1, s0 : s0 + G, :].rearrange("p s f -> p (s f)"),
                )
                axg = axp.tile([3 * 2 * T, LC], bf16, name=f"ax{rep}_{gi}", tag="ax")
                nc.gpsimd.dma_start(
                    axg[0:GP, :],
                    abx[s0 : s0 + G].rearrange("g t l -> (g t) l"),
                )
                dct = dcp.tile([P, DCW], f8, name=f"dc{rep}_{gi}", tag="dc")
                nc.gpsimd.dma_start(dct[:], dcg[gi])
                st[gi] = {"hhg": hhg, "abg": abg, "axg": axg, "dct": dct, "G": G}

            def hid(gi, j, c, n0=0, n1=H):
                hhg = st[gi]["hhg"]
                base = CHUNKS * H * j + H * c
                return hhg[:, base + n0 : base + n1]

            def hidT(gi, j, k):
                hhg = st[gi]["hhg"]
                base = CHUNKS * H * (st[gi]["G"] + j) + LC * k
                return hhg[:, base : base + LC]

            def emit_A(gi):
                # stage A: QRT = [A1|B1]^T @ h (PE, col-tiled over seqs)
                grp = GROUPS[gi]
                G = len(grp)
                GP = 32 * G
                abg = st[gi]["abg"]
                pA = pps.tile([P, H], f32, name=f"pA{rep}_{gi}", tag="p")
                for j in range(G):
                    for c in range(CHUNKS):
                        lw = abg[:, 96 * j + 32 * c : 96 * j + 32 * c + 32]
                        for n0, n1 in N_SPLITS:
                            nc.tensor.matmul(
                                pA[32 * j : 32 * j + 32, n0:n1],
                                lw,
                                hid(gi, j, c, n0, n1),
                                start=(c == 0),
                                stop=(c == CHUNKS - 1),
                            )
                # QRT -> bf16 SBUF (DVE) -> QRT^T via one xbar DMA-transpose
                sbA = sap.tile([3 * 2 * T, H], bf16, name=f"sA{rep}_{gi}", tag="sa")
                nc.vector.tensor_copy(sbA[0:GP, :], pA[0:GP, :])
                tg = tp.tile([P, HS * 3 * 2 * T], bf16, name=f"tg{rep}_{gi}", tag="tg")
                tgv = tg[:].rearrange("p (k c) -> p k c", k=HS)
                nc.sync.dma_start(tgv[:, :, 0:GP], sbA[0:GP, :], transpose=True)
                st[gi]["tg"] = tg

            def emit_Y(gi):
                # cast QRT^T to fp8, then Y' = QRT @ h^T (PE, col-tiled)
                grp = GROUPS[gi]
                G = len(grp)
                GP = 32 * G
                tgv = st[gi]["tg"][:].rearrange("p (k c) -> p k c", k=HS)
                t8 = t8p.tile([P, HS * 3 * 2 * T], f8, name=f"t8{rep}_{gi}", tag="t8")
                t8v = t8[:].rearrange("p (k c) -> p k c", k=HS)
                nc.vector.tensor_copy(t8v[:, :, 0:GP], tgv[:, :, 0:GP])
                pY = pps.tile([P, H], f32, name=f"pY{rep}_{gi}", tag="p")
                for j in range(G):
                    for k in range(HS):
                        nc.tensor.matmul(
                            pY[32 * j : 32 * j + 32, 0:LC],
                            t8[:, 96 * k + 32 * j : 96 * k + 32 * j + 32],
                            hidT(gi, j, k),
                            start=(k == 0),
                            stop=(k == HS - 1),
                        )
                st[gi]["pY"] = pY

            def emit_Zy(gi):
                # Z = abx . Y' (DVE); y cols = Z^T @ ones/16 (PE); scatter (ACT)
                grp = GROUPS[gi]
                G = len(grp)
                GP = 32 * G
                pY = st[gi]["pY"]
                axg = st[gi]["axg"]
                dct = st[gi]["dct"]
                zt = zp.tile([3 * 2 * T, LC], bf16, name=f"z{rep}_{gi}", tag="z")
                nc.vector.tensor_mul(zt[0:GP, :], pY[0:GP, 0:LC], axg[0:GP, :])
                for c in range(CHUNKS):
                    nc.tensor.matmul(
                        pY[:, 512 + 6 * c : 512 + 6 * c + 6],
                        zt[0:GP, 128 * c : 128 * c + 128],
                        onest[0:GP, :],
                        start=True,
                        stop=True,
                        skip_group_check=True,
                    )
                for c in range(CHUNKS):
                    pyv = pY[:, 512 + 6 * c : 512 + 6 * c + 6].rearrange(
                        "p (s q) -> p s q", q=2
                    )
                    if c == 0:
                        dv = dct[:, 0 : 32 * G].rearrange("p (s q) -> p s q", q=32)
                    else:
                        o = 3 * 2 * T + 12 * (c - 1)
                        dv = dct[:, o : o + 4 * G].rearrange("p (s q) -> p s q", q=4)
                    nc.scalar.copy(dv[:, :, 2:4], pyv[:, 0:G, :])

            def emit_D(gi):
                # stage D: [qs,rs,qc,rc] = [a,b,ya,yb]^T @ h (PE, col-tiled)
                grp = GROUPS[gi]
                G = len(grp)
                GP = 32 * G
                dct = st[gi]["dct"]
                pD = pps.tile([P, H], f32, name=f"pD{rep}_{gi}", tag="p")
                for j in range(G):
                    for c in range(CHUNKS):
                        if c == 0:
                            rows = 2 * T
                            lw = dct[:, 32 * j : 32 * j + 32]
                        else:
                            rows = 4
                            o = 3 * 2 * T + 12 * (c - 1)
                            lw = dct[:, o + 4 * j : o + 4 * j + 4]
                        for n0, n1 in N_SPLITS:
                            nc.tensor.matmul(
                                pD[32 * j : 32 * j + rows, n0:n1],
                                lw,
                                hid(gi, j, c, n0, n1),
                                start=(c == 0),
                                stop=(c == CHUNKS - 1),
                                skip_group_check=True,
                            )
                osb = osp.tile([3 * 2 * T, H], f32, name=f"o{rep}_{gi}", tag="o")
                nc.scalar.copy(osb[0:GP, :], pD[0:GP, :])
                # outputs ride the scalar HWDGE queue (naturally ordered
                # right after the copy, keeps the sync queue for loads)
                for j, s in enumerate(grp):
                    nc.scalar.dma_start(
                        out[4 * s : 4 * s + 4, :], osb[32 * j : 32 * j + 4, :]
                    )

            # all loads issue up front (sync queue never blocks on compute
            # sems); the software pipeline keeps the PE fed with group g+1's
            # stage A while group g's transpose round-trips through DMA
            for gi in range(NG):
                emit_loads(gi)
            emit_A(0)
            for gi in range(NG):
                if gi + 1 < NG:
                    emit_A(gi + 1)
                emit_Y(gi)
                emit_Zy(gi)
                emit_D(gi)

    nc.compile()
    return nc


def _prep_core_inputs(hidden_states, attention_mask, role_ids, turn_ids):
    """Per-core input maps: one-hot / band-smeared mask prep (index work only)."""
    import ml_dtypes

    bf16 = ml_dtypes.bfloat16
    f8 = ml_dtypes.float8_e4m3

    active = attention_mask != 0
    counts = active.sum(-1)
    assert counts.max() <= LC, f"active tokens {counts.max()} exceed LC={LC}"
    # stable-sort active tokens to the front, keep the first LC positions.
    # Padded positions carry real h values but zero masks, so every
    # contribution they could make is exactly zero.
    sel = np.argsort(~active, axis=1, kind="stable")[:, :LC]  # [B, LC]

    am = np.take_along_axis(active, sel, axis=1).astype(np.float32)
    ro = np.take_along_axis(role_ids, sel, axis=1)
    tu = np.take_along_axis(turn_ids, sel, axis=1)
    hidden_states = np.take_along_axis(hidden_states, sel[..., None], axis=1)

    a = am * (ro == 0)
    b = am * (ro == 1)
    onehot = (tu[..., None] == np.arange(T, dtype=tu.dtype)).astype(
        np.float32
    )  # [B, LC, T]
    A1 = onehot * a[..., None]
    B1 = onehot * b[..., None]
    band = (
        np.abs(np.arange(T)[:, None] - np.arange(T)[None, :]) <= VIEW_RANGE
    ).astype(np.float32)
    A1b = A1 @ band  # a_l * band[turn_l, :]
    B1b = B1 @ band

    h8 = hidden_states.astype(f8)  # [B, LC, H]
    # token-major: [B, 128, CHUNKS*H]
    hid = (
        h8.reshape(B_SEQ, CHUNKS, P, H).transpose(0, 2, 1, 3).reshape(B_SEQ, P, -1)
    )
    # H-major: [B, 128, HS*LC]
    hidT = (
        h8.transpose(0, 2, 1)
        .reshape(B_SEQ, HS, P, LC)
        .transpose(0, 2, 1, 3)
        .reshape(B_SEQ, P, -1)
    )
    # [B, 2, 128, CHUNKS*H]: index 0 = hid, 1 = hidT
    hh = np.stack([hid, hidT], axis=1)

    # stage-A weights: per chunk the [A1(16) | B1(16)] columns
    ab_full = (
        np.concatenate([A1, B1], axis=-1)
        .reshape(B_SEQ, CHUNKS, P, 2 * T)
        .transpose(0, 2, 1, 3)
        .reshape(B_SEQ, P, -1)
    ).astype(f8)

    def core_pmajor(x, c):
        # [SPC, P, F] core slice -> partition-major [P, SPC, F]
        return np.ascontiguousarray(
            x[c * SPC : (c + 1) * SPC].transpose(1, 0, 2)
        )

    abx = np.ascontiguousarray(
        np.concatenate([B1b, A1b], axis=-1).transpose(0, 2, 1)
    ).astype(bf16)  # [B, 2T, LC]

    # stage-D weight template per group: [a, b, 0, 0] cols; chunk 0 padded
    # to 32 cols so stage D's start=True initializes the full psum strip
    ab2 = np.stack([a, b], axis=-1).reshape(B_SEQ, CHUNKS, P, 2)
    dcg_all = np.zeros((N_CORES, NG, P, DCW), np.float32)
    for gi, grp in enumerate(GROUPS):
        for j, sj in enumerate(grp):
            for core in range(N_CORES):
                s = core * SPC + sj
                dcg_all[core, gi, :, 32 * j : 32 * j + 2] = ab2[s, 0]
                for c in range(1, CHUNKS):
                    o = 3 * 2 * T + 12 * (c - 1) + 4 * j
                    dcg_all[core, gi, :, o : o + 2] = ab2[s, c]

    # ones/16 pattern: for seq strip j, rows 32j..32j+16 are the b*gam half
    # (abx rows B1b x Y'_Q), rows 32j+16..32j+32 the a*gam half.
    # dct col order is [a, b, a*gam, b*gam] -> py col 2j = a*gam, 2j+1 = b*gam
    ones3 = np.zeros((3 * 2 * T, 6), np.float32)
    for j in range(3):
        ones3[32 * j + T : 32 * j + 2 * T, 2 * j] = SC  # a*gam
        ones3[32 * j : 32 * j + T, 2 * j + 1] = SC  # b*gam
    ones3 = ones3.astype(bf16)

    in_maps = []
    for c in range(N_CORES):
        sl = slice(c * SPC, (c + 1) * SPC)
        in_maps.append(
            {
                # [P, 2, SPC, CHUNKS*H]
                "hh": np.ascontiguousarray(
                    hh[c * SPC : (c + 1) * SPC].transpose(2, 1, 0, 3)
                ),
                "ab": core_pmajor(ab_full, c),
                "abx": np.ascontiguousarray(abx[sl]),
                "dcg": np.ascontiguousarray(dcg_all[c]).astype(f8),
                "ones": ones3,
            }
        )
    # cheap reference for a device-integrity check: qs/rs rows only,
    # recomputed on host in fp32 from the same fp8 values
    hf = h8.astype(np.float32)
    qs_ref = np.einsum("bl,blh->bh", a, hf)
    rs_ref = np.einsum("bl,blh->bh", b, hf)
    return in_maps, a.sum(-1), b.sum(-1), qs_ref, rs_ref


def _outputs_ok(outs, qs_ref, rs_ref):
    """Detect corrupted device runs: finite outputs + stage-D qs/rs match host."""
    vecs = np.concatenate(outs, axis=0).reshape(-1, 4, H)
    if not np.isfinite(vecs).all():
        return False
    for got, ref in ((vecs[:, 0], qs_ref), (vecs[:, 1], rs_ref)):
        num = np.linalg.norm(got - ref, axis=-1)
        den = np.linalg.norm(ref, axis=-1) + 1e-6
        if (num / den).max() > 0.05:
            return False
    return True


def _finalize(outs, labels, na, nb):
    """Host-side O(B*H) reduction: cosine, log-softmax, label-weighted loss."""
    vecs = np.concatenate(outs, axis=0).astype(np.float64).reshape(-1, 4, H)
    qs = vecs[:, 0] / (na + AVG_EPS)[:, None]
    rs = vecs[:, 1] / (nb + AVG_EPS)[:, None]
    qc = vecs[:, 2] / (nb + AVG_EPS)[:, None]  # carries the SC scale: cancels
    rc = vecs[:, 3] / (na + AVG_EPS)[:, None]

    def cos(x, y):
        nx = np.maximum(np.linalg.norm(x, axis=-1), COS_EPS)
        ny = np.maximum(np.linalg.norm(y, axis=-1), COS_EPS)
        return (x * y).sum(-1) / (nx * ny)

    logit_q = (cos(qs, qc) / TEMP).reshape(-1, SAMPLES)
    logit_r = (cos(rs, rc) / TEMP).reshape(-1, SAMPLES)

    def lsm(x):
        m = x.max(-1, keepdims=True)
        e = np.exp(x - m)
        return x - m - np.log(e.sum(-1, keepdims=True))

    lab = labels.astype(np.float64)
    loss_q = -np.mean(lsm(logit_q) * lab)
    loss_r = -np.mean(lsm(logit_r) * lab)
    return np.float32(loss_r + loss_q)


def kernel(hidden_states, labels, attention_mask, role_ids, turn_ids):
    import time

    from concourse.bass_utils import run_bass_kernel_spmd

    if "nc" not in _CACHE:
        _CACHE["nc"] = _build_nc()
    nc = _CACHE["nc"]

    in_maps, na, nb, qs_ref, rs_ref = _prep_core_inputs(
        np.asarray(hidden_states),
        np.asarray(attention_mask),
        np.asarray(role_ids),
        np.asarray(turn_ids),
    )
    trace = bool(os.environ.get("BASS_KERNEL_TRACE"))

    # the axon/NRT path very occasionally drops a run (device-unrecoverable
    # or corrupted output); validate cheaply and retry rather than fail
    outs = None
    for attempt in range(3):
        try:
            res = run_bass_kernel_spmd(
                nc, in_maps, core_ids=list(range(N_CORES)), trace=trace
            )
            cand = [res.results[c]["out"] for c in range(N_CORES)]
        except Exception:
            if attempt == 2:
                raise
            time.sleep(2.0)
            continue
        outs = cand
        if _outputs_ok(cand, qs_ref, rs_ref):
            break
    if trace:
        _CACHE["last_results"] = res
        print(
            f"[kernel] exec_time_ns={res.exec_time_ns} "
            f"mean_exec_time_ns={res.mean_exec_time_ns}"
        )
    return _finalize(outs, np.asarray(labels), na, nb)


# revision 27
# speedup vs baseline: 1.2029x; 1.0301x over previous
"""Trainium2 Bass kernel for the Dial2vec contrastive loss (nn_Dial2vec).

Math: the dense reference computes, per sequence,
    q = h * a[:,None]; r = h * b[:,None]               (a/b = role-0/1 masks)
    w = q @ r^T; fw = w * band                         (band from turn ids)
    q_cross = fw^T @ q; r_cross = fw @ r
then masked means of q / q_cross / r / r_cross, cosine similarities, and a
label-weighted log-softmax loss.

Because band[i,j] depends only on (turn_i, turn_j) and a*b == 0, everything
collapses to per-turn segment sums over the 16 turns:
    Q_T[t] = sum_{turn_l = t} a_l h_l;  R_T[t] likewise with b     [16, H]
    gam_l  = a_l (Band R_T)[turn_l].h_l + b_l (Band Q_T)[turn_l].h_l
    qs = sum a_l h_l; qc = sum a_l gam_l h_l; rs/rc likewise with b
and cosine similarity is scale-invariant, so the mask-count denominators
cancel and gam can carry an arbitrary power-of-two scale (1/16 here, to fit
fp8 range).

Device pipeline per 3-sequence group (data parallel over 8 cores, one
dialogue = 10 sequences per core, fp8 activations with fp32 PSUM accumulate):
  A  : QRT[32,H] = [A1|B1]^T @ h          (token contraction, PE col-tiled)
  T  : QRT^T via one xbar DMA-transpose   (bf16, SBUF->SBUF)
  Y' : Y[32,LC] = QRT @ h^T               (H contraction vs host-shipped
                                           H-major h, PE col-tiled)
  Z  : Z = abx . Y'                       (one small DVE multiply)
  y  : per-token gam-mask cols = Z^T @ ones/16   (one tiny PE matmul/chunk)
  D  : [qs,rs,qc,rc] = [a,b,y_a,y_b]^T @ h (token contraction, PE col-tiled)
The host performs index-only preprocessing (one-hot / band-smeared masks,
fp8 casts, both h layouts) and the final O(B*H) cosine/log-softmax
reduction over the 40 gathered fp32 vectors per core.
"""

import os

import numpy as np

B_SEQ = 80
L = 512
H = 768
SAMPLES = 10
T = 16
VIEW_RANGE = 2
TEMP = 0.2
AVG_EPS = 1e-6
COS_EPS = 1e-8

N_CORES = 8
SPC = SAMPLES  # sequences per core = one dialogue
P = 128
LC = 384  # compacted token count (attention_mask=1 tokens only, zero-padded)
CHUNKS = LC // P  # 3
HS = H // P  # 6 H-slices
N_SPLITS = ((0, 512), (512, 768))  # PSUM-bank-aligned fp32 free-dim splits
SC = 1.0 / 16  # gam scale (power of two; cancels in cosine)

# 2T-row supergroups of 3 sequences (PSUM base partitions {0,32,64})
GROUPS = [list(range(g, min(g + 3, SPC))) for g in range(0, SPC, 3)]
NG = len(GROUPS)
FSEQ = 2 * CHUNKS * H  # per-seq hh cols: token-major hid | H-major hidT
DCW = 3 * 2 * T + 4 * 3 * (CHUNKS - 1)  # dcg cols per group: 96 + 12 + 12

_CACHE: dict = {}


def _build_nc(repeat: int = 1):
    """Build + compile the per-core Bass program (identical on all cores)."""
    from contextlib import ExitStack

    import concourse.bacc as bacc
    import concourse.mybir as mybir
    import concourse.tile as tile

    f32 = mybir.dt.float32
    bf16 = mybir.dt.bfloat16
    f8 = mybir.dt.float8e4

    nc = bacc.Bacc(
        "TRN2",
        debug=False,
        enable_asserts=False,
        target_bir_lowering=False,
    )

    # partition-major layouts: a group load reads one contiguous
    # [128, G*F] block per partition -> large DMA descriptors.
    # hh dim 1: 0 = token-major hid, 1 = H-major hidT
    hh = nc.dram_tensor("hh", [P, 2, SPC, CHUNKS * H], f8, kind="ExternalInput").ap()
    # every small operand rides in ONE fp8 tensor / ONE DMA (keeps the
    # DMAHW sem-lane rotation from blocking the transposes):
    # [ab: SPC*96 | dcg: NG*DCW | abx: NG*LC | ones: 6]
    AB_W = SPC * CHUNKS * 2 * T
    AX_O = AB_W + NG * DCW
    ON_O = AX_O + NG * LC
    aux = nc.dram_tensor("aux", [P, ON_O + 6], f8, kind="ExternalInput").ap()
    out = nc.dram_tensor("out", [4 * SPC, H], f32, kind="ExternalOutput").ap()

    with tile.TileContext(nc) as tc, ExitStack() as ctx:
        hhp = ctx.enter_context(tc.tile_pool(name="hhp", bufs=NG))
        sap = ctx.enter_context(tc.tile_pool(name="sap", bufs=2))
        tp = ctx.enter_context(tc.tile_pool(name="tp", bufs=2))
        t8p = ctx.enter_context(tc.tile_pool(name="t8p", bufs=2))
        zp = ctx.enter_context(tc.tile_pool(name="zp", bufs=2))
        osp = ctx.enter_context(tc.tile_pool(name="osp", bufs=2))
        onp = ctx.enter_context(tc.tile_pool(name="onp", bufs=1))
        pps = ctx.enter_context(tc.tile_pool(name="pps", bufs=4, space="PSUM"))

        auxt = onp.tile([P, ON_O + 6], f8, name="aux", tag="aux")
        nc.gpsimd.dma_start(auxt[:], aux)

        for rep in range(repeat):
            st: dict = {}

            def emit_loads(gi):
                grp = GROUPS[gi]
                G = len(grp)
                s0 = grp[0]
                HW2 = CHUNKS * H  # per-seq half width (2304)
                hhg = hhp.tile([P, 3 * FSEQ], f8, name=f"hh{rep}_{gi}", tag="hh")
                # hid half on the sync HWDGE ring, hidT half on the scalar
                # HWDGE ring: contiguous [128, G*2304] blocks on both sides
                nc.sync.dma_start(
                    hhg[:, 0 : G * HW2],
                    hh[:, 0, s0 : s0 + G, :].rearrange("p s f -> p (s f)"),
                )
                nc.scalar.dma_start(
                    hhg[:, G * HW2 : 2 * G * HW2],
                    hh[:, 1, s0 : s0 + G, :].rearrange("p s f -> p (s f)"),
                )
                st[gi] = {"hhg": hhg, "G": G}

            def abw(s, c):
                # stage-A weights of sequence s, chunk c
                o = 96 * s + 32 * c
                return auxt[:, o : o + 32]

            def dctv(gi):
                o = AB_W + DCW * gi
                return auxt[:, o : o + DCW]

            def axgv(gi, GP):
                o = AX_O + LC * gi
                return auxt[0:GP, o : o + LC]

            def onesv(GP):
                return auxt[0:GP, ON_O : ON_O + 6]

            def hid(gi, j, c, n0=0, n1=H):
                hhg = st[gi]["hhg"]
                base = CHUNKS * H * j + H * c
                return hhg[:, base + n0 : base + n1]

            def hidT(gi, j, k):
                hhg = st[gi]["hhg"]
                base = CHUNKS * H * (st[gi]["G"] + j) + LC * k
                return hhg[:, base : base + LC]

            def emit_A(gi):
                # stage A: QRT = [A1|B1]^T @ h (PE, col-tiled over seqs)
                grp = GROUPS[gi]
                G = len(grp)
                GP = 32 * G
                pA = pps.tile([P, H], f32, name=f"pA{rep}_{gi}", tag="p")
                for j in range(G):
                    for c in range(CHUNKS):
                        lw = abw(grp[j], c)
                        for n0, n1 in N_SPLITS:
                            nc.tensor.matmul(
                                pA[32 * j : 32 * j + 32, n0:n1],
                                lw,
                                hid(gi, j, c, n0, n1),
                                start=(c == 0),
                                stop=(c == CHUNKS - 1),
                            )
                # QRT -> bf16 SBUF (DVE) -> QRT^T via one xbar DMA-transpose
                sbA = sap.tile([3 * 2 * T, H], bf16, name=f"sA{rep}_{gi}", tag="sa")
                nc.vector.tensor_copy(sbA[0:GP, :], pA[0:GP, :])
                tg = tp.tile([P, HS * 3 * 2 * T], bf16, name=f"tg{rep}_{gi}", tag="tg")
                tgv = tg[:].rearrange("p (k c) -> p k c", k=HS)
                nc.sync.dma_start(tgv[:, :, 0:GP], sbA[0:GP, :], transpose=True)
                st[gi]["tg"] = tg

            def emit_Y(gi):
                # cast QRT^T to fp8, then Y' = QRT @ h^T (PE, col-tiled)
                grp = GROUPS[gi]
                G = len(grp)
                GP = 32 * G
                tgv = st[gi]["tg"][:].rearrange("p (k c) -> p k c", k=HS)
                t8 = t8p.tile([P, HS * 3 * 2 * T], f8, name=f"t8{rep}_{gi}", tag="t8")
                t8v = t8[:].rearrange("p (k c) -> p k c", k=HS)
                nc.vector.tensor_copy(t8v[:, :, 0:GP], tgv[:, :, 0:GP])
                pY = pps.tile([P, H], f32, name=f"pY{rep}_{gi}", tag="p")
                for j in range(G):
                    for k in range(HS):
                        nc.tensor.matmul(
                            pY[32 * j : 32 * j + 32, 0:LC],
                            t8[:, 96 * k + 32 * j : 96 * k + 32 * j + 32],
                            hidT(gi, j, k),
                            start=(k == 0),
                            stop=(k == HS - 1),
                        )
                st[gi]["pY"] = pY

            def emit_Zy(gi):
                # Z = abx . Y'/16 (one fused DVE op, fp8 out);
                # y cols = Z^T @ ones (PE); scatter into aux (ACT)
                grp = GROUPS[gi]
                G = len(grp)
                GP = 32 * G
                pY = st[gi]["pY"]
                dct = dctv(gi)
                zt = zp.tile([3 * 2 * T, LC], f8, name=f"z{rep}_{gi}", tag="z")
                nc.vector.scalar_tensor_tensor(
                    zt[0:GP, :],
                    pY[0:GP, 0:LC],
                    SC,
                    axgv(gi, GP),
                    mybir.AluOpType.mult,
                    mybir.AluOpType.mult,
                )
                for c in range(CHUNKS):
                    nc.tensor.matmul(
                        pY[:, 512 + 6 * c : 512 + 6 * c + 6],
                        zt[0:GP, 128 * c : 128 * c + 128],
                        onesv(GP),
                        start=True,
                        stop=True,
                        skip_group_check=True,
                    )
                for c in range(CHUNKS):
                    pyv = pY[:, 512 + 6 * c : 512 + 6 * c + 6].rearrange(
                        "p (s q) -> p s q", q=2
                    )
                    if c == 0:
                        dv = dct[:, 0 : 32 * G].rearrange("p (s q) -> p s q", q=32)
                    else:
                        o = 3 * 2 * T + 12 * (c - 1)
                        dv = dct[:, o : o + 4 * G].rearrange("p (s q) -> p s q", q=4)
                    nc.scalar.copy(dv[:, :, 2:4], pyv[:, 0:G, :])

            def emit_D(gi):
                # stage D: [qs,rs,qc,rc] = [a,b,ya,yb]^T @ h (PE, col-tiled)
                grp = GROUPS[gi]
                G = len(grp)
                GP = 32 * G
                dct = dctv(gi)
                pD = pps.tile([P, H], f32, name=f"pD{rep}_{gi}", tag="p")
                for j in range(G):
                    for c in range(CHUNKS):
                        if c == 0:
                            rows = 2 * T
                            lw = dct[:, 32 * j : 32 * j + 32]
                        else:
                            rows = 4
                            o = 3 * 2 * T + 12 * (c - 1)
                            lw = dct[:, o + 4 * j : o + 4 * j + 4]
                        for n0, n1 in N_SPLITS:
                            nc.tensor.matmul(
                                pD[32 * j : 32 * j + rows, n0:n1],
                                lw,
                                hid(gi, j, c, n0, n1),
                                start=(c == 0),
                                stop=(c == CHUNKS - 1),
                                skip_group_check=True,
                            )
                osb = osp.tile([3 * 2 * T, H], f32, name=f"o{rep}_{gi}", tag="o")
                nc.scalar.copy(osb[0:GP, :], pD[0:GP, :])
                # outputs ride the scalar HWDGE queue (naturally ordered
                # right after the copy, keeps the sync queue for loads)
                for j, s in enumerate(grp):
                    nc.scalar.dma_start(
                        out[4 * s : 4 * s + 4, :], osb[32 * j : 32 * j + 4, :]
                    )

            # all loads issue up front (sync queue never blocks on compute
            # sems); the software pipeline keeps the PE fed with group g+1's
            # stage A while group g's transpose round-trips through DMA
            for gi in range(NG):
                emit_loads(gi)
            emit_A(0)
            for gi in range(NG):
                if gi + 1 < NG:
                    emit_A(gi + 1)
                emit_Y(gi)
                emit_Zy(gi)
                emit_D(gi)

    nc.compile()
    return nc


def _prep_core_inputs(hidden_states, attention_mask, role_ids, turn_ids):
    """Per-core input maps: one-hot / band-smeared mask prep (index work only)."""
    import ml_dtypes

    bf16 = ml_dtypes.bfloat16
    f8 = ml_dtypes.float8_e4m3

    active = attention_mask != 0
    counts = active.sum(-1)
    assert counts.max() <= LC, f"active tokens {counts.max()} exceed LC={LC}"
    # stable-sort active tokens to the front, keep the first LC positions.
    # Padded positions carry real h values but zero masks, so every
    # contribution they could make is exactly zero.
    sel = np.argsort(~active, axis=1, kind="stable")[:, :LC]  # [B, LC]

    am = np.take_along_axis(active, sel, axis=1).astype(np.float32)
    ro = np.take_along_axis(role_ids, sel, axis=1)
    tu = np.take_along_axis(turn_ids, sel, axis=1)
    hidden_states = np.take_along_axis(hidden_states, sel[..., None], axis=1)

    a = am * (ro == 0)
    b = am * (ro == 1)
    onehot = (tu[..., None] == np.arange(T, dtype=tu.dtype)).astype(
        np.float32
    )  # [B, LC, T]
    A1 = onehot * a[..., None]
    B1 = onehot * b[..., None]
    band = (
        np.abs(np.arange(T)[:, None] - np.arange(T)[None, :]) <= VIEW_RANGE
    ).astype(np.float32)
    A1b = A1 @ band  # a_l * band[turn_l, :]
    B1b = B1 @ band

    h8 = hidden_states.astype(f8)  # [B, LC, H]
    # token-major: [B, 128, CHUNKS*H]
    hid = (
        h8.reshape(B_SEQ, CHUNKS, P, H).transpose(0, 2, 1, 3).reshape(B_SEQ, P, -1)
    )
    # H-major: [B, 128, HS*LC]
    hidT = (
        h8.transpose(0, 2, 1)
        .reshape(B_SEQ, HS, P, LC)
        .transpose(0, 2, 1, 3)
        .reshape(B_SEQ, P, -1)
    )
    # [B, 2, 128, CHUNKS*H]: index 0 = hid, 1 = hidT
    hh = np.stack([hid, hidT], axis=1)

    # stage-A weights: per chunk the [A1(16) | B1(16)] columns
    ab_full = (
        np.concatenate([A1, B1], axis=-1)
        .reshape(B_SEQ, CHUNKS, P, 2 * T)
        .transpose(0, 2, 1, 3)
        .reshape(B_SEQ, P, -1)
    ).astype(f8)

    def core_pmajor(x, c):
        # [SPC, P, F] core slice -> partition-major [P, SPC, F]
        return np.ascontiguousarray(
            x[c * SPC : (c + 1) * SPC].transpose(1, 0, 2)
        )

    abx = np.ascontiguousarray(
        np.concatenate([B1b, A1b], axis=-1).transpose(0, 2, 1)
    ).astype(bf16)  # [B, 2T, LC]

    # stage-D weight template per group: [a, b, 0, 0] cols; chunk 0 padded
    # to 32 cols so stage D's start=True initializes the full psum strip
    ab2 = np.stack([a, b], axis=-1).reshape(B_SEQ, CHUNKS, P, 2)
    dcg_all = np.zeros((N_CORES, NG, P, DCW), np.float32)
    for gi, grp in enumerate(GROUPS):
        for j, sj in enumerate(grp):
            for core in range(N_CORES):
                s = core * SPC + sj
                dcg_all[core, gi, :, 32 * j : 32 * j + 2] = ab2[s, 0]
                for c in range(1, CHUNKS):
                    o = 3 * 2 * T + 12 * (c - 1) + 4 * j
                    dcg_all[core, gi, :, o : o + 2] = ab2[s, c]

    # ones pattern (scale lives in the Z op): for seq strip j, rows
    # 32j..32j+16 are the b*gam half (abx rows B1b x Y'_Q), rows
    # 32j+16..32j+32 the a*gam half.
    # dct col order is [a, b, a*gam, b*gam] -> py col 2j = a*gam, 2j+1 = b*gam
    ones3 = np.zeros((P, 6), np.float32)
    for j in range(3):
        ones3[32 * j + T : 32 * j + 2 * T, 2 * j] = 1.0  # a*gam
        ones3[32 * j : 32 * j + T, 2 * j + 1] = 1.0  # b*gam

    AB_W = SPC * CHUNKS * 2 * T
    AX_O = AB_W + NG * DCW
    ON_O = AX_O + NG * LC
    in_maps = []
    for c in range(N_CORES):
        auxm = np.zeros((P, ON_O + 6), np.float32)
        auxm[:, 0:AB_W] = (
            ab_full[c * SPC : (c + 1) * SPC]
            .astype(np.float32)
            .transpose(1, 0, 2)
            .reshape(P, AB_W)
        )
        auxm[:, AB_W:AX_O] = dcg_all[c].transpose(1, 0, 2).reshape(P, NG * DCW)
        for gi, grp in enumerate(GROUPS):
            for j, sj in enumerate(grp):
                auxm[32 * j : 32 * j + 32, AX_O + LC * gi : AX_O + LC * (gi + 1)] = (
                    abx[c * SPC + sj]
                )
        auxm[:, ON_O:] = ones3
        in_maps.append(
            {
                # [P, 2, SPC, CHUNKS*H]
                "hh": np.ascontiguousarray(
                    hh[c * SPC : (c + 1) * SPC].transpose(2, 1, 0, 3)
                ),
                "aux": auxm.astype(f8),
            }
        )
    # cheap reference for a device-integrity check: qs/rs rows only,
    # recomputed on host in fp32 from the same fp8 values
    hf = h8.astype(np.float32)
    qs_ref = np.einsum("bl,blh->bh", a, hf)
    rs_ref = np.einsum("bl,blh->bh", b, hf)
    return in_maps, a.sum(-1), b.sum(-1), qs_ref, rs_ref


def _outputs_ok(outs, qs_ref, rs_ref):
    """Detect corrupted device runs: finite outputs + stage-D qs/rs match host."""
    vecs = np.concatenate(outs, axis=0).reshape(-1, 4, H)
    if not np.isfinite(vecs).all():
        return False
    for got, ref in ((vecs[:, 0], qs_ref), (vecs[:, 1], rs_ref)):
        num = np.linalg.norm(got - ref, axis=-1)
        den = np.linalg.norm(ref, axis=-1) + 1e-6
        if (num / den).max() > 0.05:
            return False
    return True


def _finalize(outs, labels, na, nb):
    """Host-side O(B*H) reduction: cosine, log-softmax, label-weighted loss."""
    vecs = np.concatenate(outs, axis=0).astype(np.float64).reshape(-1, 4, H)
    qs = vecs[:, 0] / (na + AVG_EPS)[:, None]
    rs = vecs[:, 1] / (nb + AVG_EPS)[:, None]
    qc = vecs[:, 2] / (nb + AVG_EPS)[:, None]  # carries the SC scale: cancels
    rc = vecs[:, 3] / (na + AVG_EPS)[:, None]

    def cos(x, y):
        nx = np.maximum(np.linalg.norm(x, axis=-1), COS_EPS)
        ny = np.maximum(np.linalg.norm(y, axis=-1), COS_EPS)
        return (x * y).sum(-1) / (nx * ny)

    logit_q = (cos(qs, qc) / TEMP).reshape(-1, SAMPLES)
    logit_r = (cos(rs, rc) / TEMP).reshape(-1, SAMPLES)

    def lsm(x):
        m = x.max(-1, keepdims=True)
        e = np.exp(x - m)
        return x - m - np.log(e.sum(-1, keepdims=True))

    lab = labels.astype(np.float64)
    loss_q = -np.mean(lsm(logit_q) * lab)
    loss_r = -np.mean(lsm(logit_r) * lab)
    return np.float32(loss_r + loss_q)


def kernel(hidden_states, labels, attention_mask, role_ids, turn_ids):
    import time

    from concourse.bass_utils import run_bass_kernel_spmd

    if "nc" not in _CACHE:
        _CACHE["nc"] = _build_nc()
    nc = _CACHE["nc"]

    in_maps, na, nb, qs_ref, rs_ref = _prep_core_inputs(
        np.asarray(hidden_states),
        np.asarray(attention_mask),
        np.asarray(role_ids),
        np.asarray(turn_ids),
    )
    trace = bool(os.environ.get("BASS_KERNEL_TRACE"))

    # the axon/NRT path very occasionally drops a run (device-unrecoverable
    # or corrupted output); validate cheaply and retry rather than fail
    outs = None
    for attempt in range(3):
        try:
            res = run_bass_kernel_spmd(
                nc, in_maps, core_ids=list(range(N_CORES)), trace=trace
            )
            cand = [res.results[c]["out"] for c in range(N_CORES)]
        except Exception:
            if attempt == 2:
                raise
            time.sleep(2.0)
            continue
        outs = cand
        if _outputs_ok(cand, qs_ref, rs_ref):
            break
    if trace:
        _CACHE["last_results"] = res
        print(
            f"[kernel] exec_time_ns={res.exec_time_ns} "
            f"mean_exec_time_ns={res.mean_exec_time_ns}"
        )
    return _finalize(outs, np.asarray(labels), na, nb)
